# revision 123
# baseline (speedup 1.0000x reference)
"""Multi-head causal attention with interleaved RoPE on 8 Trainium2 cores.

nn_MultiHeadAttention: x[4,2048,1024], W_qkv[3072,1024], W_o[1024,1024],
16 heads x d_k=64, interleaved RoPE, causal softmax.

Sharding: core c = 2*b + g handles batch b (of 4) and head-group g (of 2,
8 heads each). Each core computes a full-width partial output for its batch
(o_heads @ W_o[:, group-cols]); the host sums the two partials per batch
(the "all-reduce after o_proj", done on host at gather time).

Device strategy (per core):
 - merged projection pass: each x strip (loaded once, per-db [128,512]
   tiles) feeds q/k projection + RoPE AND v projection.
 - bf16 attention data path (qrot/krot/pt/vaug/ot) so every score/PV matmul
   runs 1 cyc/row even on sub-256-column causal edge pieces; projections in
   fp32r.
 - RoPE: ACT copies the qk PSUM to bf16 (qtmp), PE applies a 32-row-swap
   permutation matmul (psw), DVE does t1=qtmp*cos (bf16 2x), t2=psw*sin,
   dst=t1+t2 with a lag-1 software pipeline.
 - scores transposed: S^T[k,q] = k_rot . q_rot per head; exp on ACT with
   1/sqrt(dk) folded into the activation scale; causal = block skipping +
   bf16 multiplicative mask on the diagonal 128x128 block post-exp.
 - PV with lhsT = [v | ones]: softmax denominator falls out as PSUM row 64;
   normalize produces o^T bf16 = exactly o_proj's lhsT.
 - emission order interleaves phases so the scheduler overlaps them:
   [proj st0,1] [attn q2=0] [proj st2,3] [attn q2=1 + o_proj sb0-7]
   [o_proj sb8-15]; PSUM pools are scoped so every section fits in 8 banks.
"""

import numpy as np
from contextlib import ExitStack

NUM_HEADS = 16
D_K = 64
THETA = 10000.0
BS, S, D = 4, 2048, 1024
N_CORES = 8
HPC = NUM_HEADS // 2          # heads per core = 8
DG = HPC * D_K                # per-core head width = 512
QT2 = 1024                    # q tile (2 PSUM banks)

_compiled = None


def _build_program():
    import concourse.mybir as mybir
    import concourse.tile as tile
    from concourse import bacc

    F32 = mybir.dt.float32
    FR = mybir.dt.float32r
    BF = mybir.dt.bfloat16
    AF = mybir.ActivationFunctionType

    nc = bacc.Bacc("TRN2", target_bir_lowering=False, debug=False,
                   num_devices=N_CORES)

    xt_d = nc.dram_tensor("xt", [D, S], BF, kind="ExternalInput")
    wqkvt_d = nc.dram_tensor("wqkvt", [D, 3 * DG], BF,
                              kind="ExternalInput")
    wot_d = nc.dram_tensor("wot", [DG, D], BF, kind="ExternalInput")
    perm_d = nc.dram_tensor("perm", [128, 128], BF, kind="ExternalInput")
    cos_d = nc.dram_tensor("cost", [128, S], BF, kind="ExternalInput")
    sin_d = nc.dram_tensor("sint", [128, S], F32, kind="ExternalInput")
    out_d = nc.dram_tensor("out", [S, D], BF, kind="ExternalOutput")

    n_sb = S // 128           # 16 s-blocks
    n_st = S // 512           # 4 s-tiles
    n_db = D // 128           # 8 d-blocks
    inv_sqrt_dk = 1.0 / float(np.sqrt(D_K))
    # Schraudolph fast-exp in bf16 bits: bf16_bits(exp(s/8)) ~=
    # s * (128 * log2(e) / 8) + (127 << 7) - 7.4
    EXPA = 128.0 * 1.4426950408889634 / 8.0
    EXPB = 16256.0 - 7.4

    with tile.TileContext(nc) as tc, ExitStack() as octx:
        OP = octx.enter_context
        # ---------- persistent SBUF pools ----------
        qk_p = OP(tc.tile_pool(name="qk", bufs=1))
        qrot = [[qk_p.tile([128, 512], BF, tag=f"qr{t}_{st}",
                           name=f"qr{t}_{st}") for st in range(n_st)]
                for t in range(4)]
        krot = [[qk_p.tile([128, 512], BF, tag=f"kr{t}_{st}",
                           name=f"kr{t}_{st}") for st in range(n_st)]
                for t in range(4)]
        wot_p = OP(tc.tile_pool(name="wotp", bufs=1))
        wot = [wot_p.tile([128, D], BF, tag=f"wot{i}", name=f"wott{i}")
               for i in range(4)]
        const_p = OP(tc.tile_pool(name="amisc", bufs=1))
        # bf16 causal mask for the S^T diagonal block: 1 if k <= q else 0
        dmask = const_p.tile([128, 128], BF)
        nc.gpsimd.memset(dmask[:], 1.0)
        nc.gpsimd.affine_select(
            out=dmask[:], in_=dmask[:],
            compare_op=mybir.AluOpType.is_ge, fill=0.0, base=0,
            pattern=[[1, 128]], channel_multiplier=-1,
        )
        vaug_p = OP(tc.tile_pool(name="vaug", bufs=1))
        vaug = [vaug_p.tile([128, HPC * (D_K + 1)], BF, tag=f"va{i}",
                            name=f"va{i}") for i in range(n_sb)]
        for sb in range(n_sb):
            ones_view = vaug[sb][:].rearrange("p (h c) -> p h c", c=D_K + 1)
            nc.gpsimd.memset(ones_view[:, :, D_K:D_K + 1], 1.0)
        ot_p = OP(tc.tile_pool(name="otp", bufs=1))
        otq = [[ot_p.tile([128, QT2], BF, tag=f"ot{h2}_{t}",
                          name=f"ot{h2}_{t}") for t in range(4)]
               for h2 in range(2)]
        cs_p = OP(tc.tile_pool(name="cs", bufs=1))
        xt_p = OP(tc.tile_pool(name="xtp", bufs=8))
        w_p = OP(tc.tile_pool(name="w", bufs=1))
        qtmp_p = OP(tc.tile_pool(name="qtmp", bufs=5))
        rot_p = OP(tc.tile_pool(name="rot", bufs=5))
        pt_p = OP(tc.tile_pool(name="pt", bufs=3))
        nrm_p = OP(tc.tile_pool(name="nrm", bufs=2))
        outs_p = OP(tc.tile_pool(name="outs", bufs=4))

        # ---------- manually scoped PSUM pools ----------
        projA_cm = tc.tile_pool(name="ppA", bufs=4, space="PSUM")
        projA = projA_cm.__enter__()
        projB_cm = tc.tile_pool(name="ppB", bufs=4, space="PSUM")
        projB = projB_cm.__enter__()

        wqk_lo = [w_p.tile([128, DG], BF, tag=f"wqkl{i}", name=f"wqkl{i}")
                  for i in range(n_db)]
        wqk_hi_t = w_p.tile([128, n_db * DG], BF, tag="wqkh", name="wqkh")
        wv_t = w_p.tile([128, n_db * DG], BF, tag="wvt", name="wvt")
        wv = [wv_t[:, db * DG:(db + 1) * DG] for db in range(n_db)]
        perm_t = cs_p.tile([128, 128], BF, name="perm_t")
        cos_t = cs_p.tile([128, S], BF)
        sin_t = cs_p.tile([128, S], F32)

        xts = {}  # (st, db) -> [128, 512] bf16 view

        def load_strip_db2(st, db, eng=None):
            """one DMA covering db and db+1 chunks."""
            t = xt_p.tile([128, 1024], BF, tag="xts", bufs=8,
                          name=f"x{st}_{db}")
            src = xt_d.ap().rearrange("(db p) s -> p db s", p=128)
            (eng or nc.sync).dma_start(
                t[:].rearrange("p (db s) -> p db s", db=2),
                src[:, db:db + 2, st * 512:(st + 1) * 512])
            xts[(st, db)] = t[:, 0:512]
            xts[(st, db + 1)] = t[:, 512:1024]

        def load_strip_whole(st):
            t = xt_p.tile([128, n_db * 512], BF, tag="xtw", bufs=2,
                          name=f"x{st}")
            src = xt_d.ap().rearrange("(db p) s -> p db s", p=128)
            nc.sync.dma_start(
                t[:].rearrange("p (db s) -> p db s", db=n_db),
                src[:, :, st * 512:(st + 1) * 512])
            for db in range(n_db):
                xts[(st, db)] = t[:, db * 512:(db + 1) * 512]

        # DMA emission order = SP-queue order: stream what the PE needs
        # soonest; lo/strip0 chunked per-db so matmuls start on first
        # arrival, the rest as single descriscattered DMAs (HWDGE issue is
        # 625ns each and serializes - count matters more than granularity).
        for db in range(0, n_db, 2):
            nc.sync.dma_start(
                wqk_lo[db][:].rearrange("p (g c) -> p g c", g=1),
                wqkvt_d.ap().rearrange("(db p) c -> p db c", p=128)
                [:, db:db + 1, 0:512])
            nc.sync.dma_start(
                wqk_lo[db + 1][:].rearrange("p (g c) -> p g c", g=1),
                wqkvt_d.ap().rearrange("(db p) c -> p db c", p=128)
                [:, db + 1:db + 2, 0:512])
            load_strip_db2(0, db, eng=nc.scalar if db < 4 else None)
            if db == 0:
                nc.sync.dma_start(perm_t[:], perm_d.ap())
        nc.sync.dma_start(
            wqk_hi_t[:].rearrange("p (db c) -> p db c", db=n_db),
            wqkvt_d.ap().rearrange("(db p) c -> p db c", p=128)[:, :,
                                                               512:1024])
        nc.sync.dma_start(
            wv_t[:].rearrange("p (db c) -> p db c", db=n_db),
            wqkvt_d.ap().rearrange("(db p) c -> p db c", p=128)[:, :,
                                                               1024:1536])
        nc.sync.dma_start(cos_t[:], cos_d.ap())
        nc.sync.dma_start(sin_t[:], sin_d.ap())
        for db in range(0, n_db, 2):
            load_strip_db2(1, db)
        for st in (2, 3):
            load_strip_whole(st)
        for t in range(4):
            nc.sync.dma_start(wot[t][:], wot_d.ap()[t * 128:(t + 1) * 128, :])

        def wqk_slice(db, eb):
            if eb < 4:
                return wqk_lo[db][:, eb * 128:(eb + 1) * 128]
            return wqk_hi_t[:, db * DG + (eb - 4) * 128:
                            db * DG + (eb - 3) * 128]

        rope_pending = [None]

        def psum_tile(pool):
            if pool[1] is None:
                return pool[0].tile([128, 512], F32, tag="pp", name="pp")
            return pool[0].tile([128, 512], F32, tag=pool[1],
                                bufs=3, name="pp")

        def rope_phase2(pools):
            """perm-matmul + t2 + add for the previous block (lag-1 so the
            swap matmul does not head-of-line-block the PE queue)."""
            state = rope_pending[0]
            if state is None:
                return
            rope_pending[0] = None
            qtmp, t1, dst, sl = state
            psw = psum_tile(pools[1])
            nc.tensor.matmul(psw[:], perm_t[:], qtmp[:], start=True,
                             stop=True)
            t2 = rot_p.tile([128, 512], BF, tag="t2", name="t2")
            nc.vector.tensor_mul(t2[:], psw[:], sin_t[:, sl])
            nc.vector.tensor_add(dst[:], t1[:], t2[:])

        def emit_proj_eb_mm(st, eb, pools):
            ps = psum_tile(pools[0])
            for db in range(n_db):
                nc.tensor.matmul(
                    ps[:], wqk_slice(db, eb), xts[(st, db)][:],
                    start=(db == 0), stop=(db == n_db - 1))
            return ps

        def emit_proj_eb_post(st, eb, ps, pools):
            sl = slice(st * 512, (st + 1) * 512)
            dst = (qrot if eb < 4 else krot)[eb % 4][st]
            qtmp = qtmp_p.tile([128, 512], BF, tag="qtmp")
            nc.scalar.copy(qtmp[:], ps[:])
            t1 = rot_p.tile([128, 512], BF, tag="t1")
            nc.vector.tensor_mul(t1[:], qtmp[:], cos_t[:, sl])
            rope_phase2(pools)
            rope_pending[0] = (qtmp, t1, dst, sl)

        def emit_proj_eb(st, eb, pools):
            ps = emit_proj_eb_mm(st, eb, pools)
            emit_proj_eb_post(st, eb, ps, pools)

        def emit_proj_v(st, j, pools):
            sb = st * 4 + j
            vps = psum_tile(pools[0])
            for db in range(n_db):
                nc.tensor.matmul(
                    vps[:], xts[(st, db)][:, j * 128:(j + 1) * 128],
                    wv[db][:],
                    start=(db == 0), stop=(db == n_db - 1))
            src = vps[:].rearrange("p (h c) -> p h c", c=D_K)
            dst_v = vaug[sb][:].rearrange("p (h c) -> p h c", c=D_K + 1)
            nc.scalar.copy(dst_v[:, :, 0:D_K], src)

        def emit_proj_st(st, pools, order=None):
            if order is None:
                order = [("eb", i) for i in range(8)] +                         [("v", j) for j in range(4)]
            for kind, i in order:
                if kind == "eb":
                    emit_proj_eb(st, i, pools)
                else:
                    emit_proj_v(st, i, pools)

        # ---- section 1: proj st0, st1 (6 proj PSUM banks via A+B) ----
        pA, pB = (projA, None), (projB, None)
        emit_proj_st(0, (pA, pB))
        emit_proj_st(1, (pB, pA))
        rope_phase2((pA, pB))
        projB_cm.__exit__(None, None, None)
        projA_cm.__exit__(None, None, None)

        # shared attention/proj-tail/o_proj PSUM pool: tag "sc" 2x2 banks +
        # tag "ot" 2x2 banks = 8 banks
        aps_cm = tc.tile_pool(name="aps", bufs=2, space="PSUM")
        aps = aps_cm.__enter__()
        sps_p = aps
        opsA = aps

        def qr_piece(ti, po, q0, a, b):
            qa = q0 + a
            sti, loc = qa // 512, qa % 512
            return qrot[ti][sti][po:po + 64, loc:loc + (b - a)]

        def emit_attn_head(q2, h, ops_pool, fillers=None, dve_extra=False):
            ti, po = h // 2, (h % 2) * 64
            vlo = h * (D_K + 1)
            q0 = q2 * QT2
            kb_end = (q0 + QT2) // 128
            kb_last0 = q0 // 128 + 3      # last kb writing bank 0
            opsh = [ops_pool.tile([D_K + 1, 512], F32, tag="ot", bufs=3,
                                  name="opsh") for _ in range(2)]

            def emit_pv(kb, pts):
                c0 = max(0, kb * 128 - q0)
                if c0 < 512:
                    nc.tensor.matmul(
                        opsh[0][:, c0:512], vaug[kb][:, vlo:vlo + D_K + 1],
                        pts[0][:, c0:512],
                        start=(kb == 0), stop=(kb == kb_last0))
                b1 = max(c0, 512)
                nc.tensor.matmul(
                    opsh[1][:, b1 - 512:512],
                    vaug[kb][:, vlo:vlo + D_K + 1],
                    pts[1][:, b1 - 512:512],
                    start=(kb == 0), stop=(kb == kb_end - 1))

            pend_pv = None
            for kb in range(kb_end):
                c0 = max(0, kb * 128 - q0)
                klhs = krot[ti][kb // 4][po:po + 64,
                                         (kb % 4) * 128:(kb % 4 + 1) * 128]
                pts = {}
                for hf in (0, 1):
                    lo, hi = hf * 512, (hf + 1) * 512
                    a = max(c0, lo)
                    if a >= hi:
                        continue
                    la = a - lo
                    sc = sps_p.tile([128, 512], F32, tag="sc", bufs=5,
                                    name="sc")
                    nc.tensor.matmul(sc[:, la:512], klhs,
                                     qr_piece(ti, po, q0, a, hi),
                                     start=True, stop=True)
                    pt = pt_p.tile([128, 512], BF, tag="pt", bufs=16,
                                   name="pt")
                    if q2 == 1 and kb % 2 == 0 and hf == kb % 4 // 2:
                        nc.vector.tensor_scalar(
                            pt[:, la:512].bitcast(mybir.dt.int16),
                            sc[:, la:512], EXPA, EXPB,
                            mybir.AluOpType.mult, mybir.AluOpType.add)
                    else:
                        nc.scalar.activation(pt[:, la:512], sc[:, la:512],
                                             AF.Exp, scale=inv_sqrt_dk)
                    if kb * 128 >= q0 and lo <= c0 < hi:
                        nc.vector.tensor_mul(pt[:, c0 - lo:c0 - lo + 128],
                                             pt[:, c0 - lo:c0 - lo + 128],
                                             dmask[:])
                    pts[hf] = pt
                if pend_pv is not None:
                    emit_pv(*pend_pv)
                pend_pv = (kb, pts)
                if kb in (0, 8) and fillers is not None:
                    f = next(fillers, None)
                    if f is not None:
                        f()
            emit_pv(*pend_pv)
            # normalize: o^T[dv, q] / den[q]; reciprocal reads PSUM row 64.
            # Done in two pipelined 512-col halves to shorten the
            # recip->broadcast->mul latency chain at head boundaries.
            rinv = nrm_p.tile([1, QT2], F32, tag="rinv")
            den = nrm_p.tile([64, QT2], F32, tag="den")
            onrm = (None if po == 0
                    else nrm_p.tile([64, QT2], BF, tag="onrm"))
            for hf in range(2):
                sl = slice(hf * 512, (hf + 1) * 512)
                nc.vector.reciprocal(rinv[:, sl], opsh[hf][D_K:D_K + 1, :])
                nc.gpsimd.partition_broadcast(den[:, sl], rinv[:, sl])
                if po == 0:
                    nc.vector.tensor_mul(otq[q2][ti][0:64, sl],
                                         opsh[hf][0:D_K, :], den[:, sl])
                else:
                    nc.vector.tensor_mul(onrm[:, sl], opsh[hf][0:D_K, :],
                                         den[:, sl])
            if po != 0:
                nc.sync.dma_start(otq[q2][ti][64:128, :], onrm[:])

        def emit_oproj_sb(sb, tag="ot", split_dma=False):
            h2, lb = sb // 8, sb % 8
            lsl = slice(lb * 128, (lb + 1) * 128)
            ssl = slice(sb * 128, (sb + 1) * 128)
            ostage = outs_p.tile([128, D], BF, tag="ostage")
            for eh in range(2):
                esl = slice(eh * 512, (eh + 1) * 512)
                if tag == "sc":
                    ps = sps_p.tile([128, 512], F32, tag="sc", bufs=5,
                                    name="ps")
                else:
                    ps = sps_p.tile([128, 512], F32, tag=tag, bufs=3,
                                    name="ps")
                for t in range(4):
                    nc.tensor.matmul(ps[:], otq[h2][t][:, lsl],
                                     wot[t][:, esl],
                                     start=(t == 0), stop=(t == 3))
                nc.vector.tensor_copy(ostage[:, esl], ps[:])
                if split_dma:
                    nc.sync.dma_start(out_d.ap()[ssl, esl], ostage[:, esl])
            if not split_dma:
                nc.sync.dma_start(out_d.ap()[ssl, :], ostage[:])

        # ---- section 2: attn q2=0 (ACT-heavy) + proj st2,3 (PE-heavy),
        # emission-interleaved so the shared "sc" ring rotates between both
        scp = (aps, "ot")

        def filler_units():
            for st in (2, 3):
                for j in range(4):
                    yield lambda st=st, j=j: emit_proj_v(st, j, (scp, scp))
                for eb in range(8):
                    yield lambda st=st, eb=eb: emit_proj_eb(
                        st, eb, (scp, scp))

        fill_it = filler_units()
        for h in range(HPC):
            emit_attn_head(0, h, opsA, fillers=fill_it)
            take = 2 if h < 7 else 24
            for _ in range(take):
                f = next(fill_it, None)
                if f is not None:
                    f()
        rope_phase2((scp, scp))

        # ---- section 3: attn q2=1 (+ o_proj sb0-7 fills PE gaps) ----
        # po=64 heads first: the last head then writes ot directly (no
        # onrm DMA hop on the tail critical path)
        def oproj_units():
            for sb in range(8):
                yield lambda sb=sb: emit_oproj_sb(sb)

        op_it = oproj_units()
        for i, h in enumerate((1, 3, 5, 7, 0, 2, 4, 6)):
            emit_attn_head(1, h, opsA, fillers=op_it if i > 0 else None,
                           dve_extra=False)
        for f in op_it:
            f()
        # ---- section 4: o_proj tail (sc ring: idle by now, and not
        # coupled to the last head's norm via the ot ring) ----
        for sb in range(8, n_sb):
            emit_oproj_sb(sb, tag="sc", split_dma=(sb >= n_sb - 4))

        aps_cm.__exit__(None, None, None)

    nc.compile()
    return nc


def _perm128():
    """[128,128] permutation: out = P.T @ x swaps 32-row halves within
    each 64-row group. P[k, m] = 1 iff k == swap(m)."""
    p = np.zeros((128, 128), np.float32)
    for m in range(128):
        k = m + 32 if (m % 64) < 32 else m - 32
        p[k, m] = 1.0
    return p


def _rope_tables(token_positions):
    pos = np.asarray(token_positions).astype(np.float32)
    half = D_K // 2
    inv_freq = (THETA ** (-np.arange(half, dtype=np.float32) * 2.0 / D_K))
    ang = pos[None, :].astype(np.float32) * inv_freq[:, None]     # [32, S]
    cos = np.cos(ang).astype(np.float32)
    sin = np.sin(ang).astype(np.float32)
    cos128 = np.tile(cos, (4, 1))                                 # [128, S]
    sin128 = np.empty((128, pos.shape[0]), np.float32)
    for g in range(4):
        sgn = -1.0 if (g % 2 == 0) else 1.0
        sin128[g * 32:(g + 1) * 32] = sgn * sin
    return np.ascontiguousarray(cos128), np.ascontiguousarray(sin128)


def kernel(x, W_qkv, W_o, token_positions):
    out, _ = _kernel_impl(x, W_qkv, W_o, token_positions, trace=False)
    return out


def _kernel_impl(x, W_qkv, W_o, token_positions, trace=False):
    global _compiled
    import ml_dtypes
    from concourse.bass_utils import run_bass_kernel_spmd

    BF_NP = ml_dtypes.bfloat16
    x = np.asarray(x, dtype=np.float32)
    W_qkv = np.asarray(W_qkv, dtype=np.float32)
    W_o = np.asarray(W_o, dtype=np.float32)

    if _compiled is None:
        _compiled = _build_program()
    nc = _compiled

    cos128, sin128 = _rope_tables(token_positions)
    perm = np.concatenate([np.arange(0, D_K, 2), np.arange(1, D_K, 2)])

    in_maps = []
    for c in range(N_CORES):
        b, g = divmod(c, 2)
        heads = range(g * HPC, (g + 1) * HPC)
        qrows = np.concatenate(
            [W_qkv[h * D_K:(h + 1) * D_K][perm] for h in heads])
        krows = np.concatenate(
            [W_qkv[D + h * D_K:D + (h + 1) * D_K][perm] for h in heads])
        vrows = np.concatenate(
            [W_qkv[2 * D + h * D_K:2 * D + (h + 1) * D_K] for h in heads])
        wqkvt = np.ascontiguousarray(
            np.concatenate([qrows, krows, vrows]).T)              # [1024,1536]
        wot = np.ascontiguousarray(W_o[:, g * DG:(g + 1) * DG].T)  # [512,1024]
        in_maps.append({
            "xt": np.ascontiguousarray(x[b].T).astype(BF_NP),
            "wqkvt": wqkvt.astype(BF_NP),
            "wot": wot.astype(BF_NP),
            "perm": _perm128().astype(BF_NP),
            "cost": cos128.astype(BF_NP),
            "sint": sin128,
        })

    res = run_bass_kernel_spmd(nc, in_maps, list(range(N_CORES)), trace=trace)
    out = np.empty((BS, S, D), dtype=np.float32)
    for b in range(BS):
        out[b] = (res.results[2 * b]["out"].astype(np.float32)
                  + res.results[2 * b + 1]["out"].astype(np.float32))
    return out, res.exec_time_ns
